# revision 4
# baseline (speedup 1.0000x reference)
"""Trainium2 Bass kernel for GQA attention layer (RoPE + causal + GQA 32q/8kv).

Self-contained: hardcodes shapes from the problem spec.
  hidden_states [2, 2048, 4096] f32, positions [2, 2048] i32,
  Wq [4096, 4096], Wk [1024, 4096], Wv [1024, 4096], Wo [4096, 4096]  (all f32)
Sharding: tensor-parallel over heads across 8 cores. Core c gets kv head c and
q heads 4c..4c+3. Each core computes its partial Wo output; host sums partials.
"""

import math
import os
import sys
import types
import contextlib

import numpy as np
import ml_dtypes

BF16NP = ml_dtypes.bfloat16

# ---- problem constants (hardcoded per spec) ----
P = 128
B = 2
S = 2048            # tokens per batch
HID = 4096
NH, NKV, HD = 32, 8, 128
NCORES = 8
HPC = NH // NCORES  # q heads per core (4)
T = B * S
SCALE = 1.0 / math.sqrt(HD)
ROPE_BASE = 10000.0

LAST = {}           # exec_time_ns etc from the most recent run


def _install_ntff_hook():
    """Register the axon NTFF profiling hook (image's antenv lacks axon_hooks)."""
    if "antenv.axon_hooks" in sys.modules:
        return
    try:
        import antenv
        mod = types.ModuleType("antenv.axon_hooks")
        _box = [None]
        mod.set_axon_ntff_profile_hook = lambda h: _box.__setitem__(0, h)
        mod.get_axon_ntff_profile_hook = lambda: _box[0]
        sys.modules["antenv.axon_hooks"] = mod
        antenv.axon_hooks = mod
        from trn_agent_boot.trn_boot import _ntff_profile_via_ctypes
        mod.set_axon_ntff_profile_hook(
            _ntff_profile_via_ctypes("/opt/axon/libaxon_pjrt.so")
        )
    except Exception:
        pass


def build_graph(S_=S, HID_=HID, CH=256, QC=512):
    """Build the per-core graph (identical on all 8 cores; SPMD via inputs).

    S_: tokens per batch, HID_: hidden size, CH: projection token chunk,
    QC: attention query chunk.
    """
    import concourse.bacc as bacc
    import concourse.mybir as mybir
    import concourse.tile as tile
    from contextlib import ExitStack

    BF = mybir.dt.bfloat16
    F32 = mybir.dt.float32
    Exp = mybir.ActivationFunctionType.Exp

    NKK = HID_ // P          # contraction tiles over hidden
    NCH = S_ // CH           # proj chunks per batch
    NQC = S_ // QC           # attention q chunks per batch
    NST = QC // P            # q subtiles per chunk
    NKT = S_ // P            # k tiles per batch
    NVS = CH // P            # v row-subtiles per proj chunk
    HOC = max(1, HID_ // 512)  # output column chunks
    OCW = min(512, HID_)       # output chunk width

    nc = bacc.Bacc(None)
    xT_h = nc.declare_dram_parameter("xT", [HID_, B * S_], BF, isOutput=False)
    wq_h = nc.declare_dram_parameter("wqT", [HID_, HPC * HD], BF, isOutput=False)
    wk_h = nc.declare_dram_parameter("wkT", [HID_, HD], BF, isOutput=False)
    wv_h = nc.declare_dram_parameter("wvT", [HID_, HD], BF, isOutput=False)
    wo_h = nc.declare_dram_parameter("woT", [HPC * HD, HID_], BF, isOutput=False)
    cos_h = nc.declare_dram_parameter("cos2", [P, B * S_], BF, isOutput=False)
    sin_h = nc.declare_dram_parameter("sin2", [P, B * S_], BF, isOutput=False)
    mneg_h = nc.declare_dram_parameter("mneg", [P, P], F32, isOutput=False)
    iden_h = nc.declare_dram_parameter("iden", [P, P], BF, isOutput=False)
    out_h = nc.declare_dram_parameter("out", [B * S_, HID_], BF, isOutput=True)

    xT_r = xT_h[:, :].rearrange("(ko ki) s -> ki ko s", ki=P)
    wq_r = wq_h[:, :].rearrange("(ko ki) d -> ki ko d", ki=P)
    wk_r = wk_h[:, :].rearrange("(ko ki) d -> ki ko d", ki=P)
    wv_r = wv_h[:, :].rearrange("(ko ki) d -> ki ko d", ki=P)
    wo_r = wo_h[:, :].rearrange("(oo oi) h -> oi oo h", oi=P)

    with tile.TileContext(nc) as tc, ExitStack() as ctx:
        wpool = ctx.enter_context(tc.tile_pool(name="wpool", bufs=1))
        qpool = ctx.enter_context(tc.tile_pool(name="qpool", bufs=2))
        kpool = ctx.enter_context(tc.tile_pool(name="kpool", bufs=2))
        vpool = ctx.enter_context(tc.tile_pool(name="vpool", bufs=2))
        ctpool = ctx.enter_context(tc.tile_pool(name="ctpool", bufs=2))
        xpool = ctx.enter_context(tc.tile_pool(name="xpool", bufs=2))
        wopool = ctx.enter_context(tc.tile_pool(name="wopool", bufs=2))
        ppool = ctx.enter_context(tc.tile_pool(name="ppool", bufs=3))
        rpool = ctx.enter_context(tc.tile_pool(name="rpool", bufs=3))
        cnpool = ctx.enter_context(tc.tile_pool(name="cnpool", bufs=3))
        opool = ctx.enter_context(tc.tile_pool(name="opool", bufs=3))
        scpool = ctx.enter_context(tc.tile_pool(name="scpool", bufs=4))

        psA = ctx.enter_context(tc.tile_pool(name="psA", bufs=1, space="PSUM"))
        psV = ctx.enter_context(tc.tile_pool(name="psV", bufs=1, space="PSUM"))
        psS = ctx.enter_context(tc.tile_pool(name="psS", bufs=2, space="PSUM"))
        psC = ctx.enter_context(tc.tile_pool(name="psC", bufs=4, space="PSUM"))

        # --- persistent weights / tables ---
        wq_sb = wpool.tile([P, NKK, HPC * HD], BF)
        nc.sync.dma_start(out=wq_sb, in_=wq_r)
        wk_sb = wpool.tile([P, NKK, HD], BF)
        nc.sync.dma_start(out=wk_sb, in_=wk_r)
        wv_sb = wpool.tile([P, NKK, HD], BF)
        nc.sync.dma_start(out=wv_sb, in_=wv_r)
        cos_sb = wpool.tile([P, B * S_], BF)
        nc.sync.dma_start(out=cos_sb, in_=cos_h[:, :])
        sin_sb = wpool.tile([P, B * S_], BF)
        nc.sync.dma_start(out=sin_sb, in_=sin_h[:, :])
        mneg_sb = wpool.tile([P, P], F32)
        nc.sync.dma_start(out=mneg_sb, in_=mneg_h[:, :])
        iden_sb = wpool.tile([P, P], BF)
        nc.sync.dma_start(out=iden_sb, in_=iden_h[:, :])

        def rope(ps, dst, c0, c1):
            """Neox RoPE on [128 d, n] tile: rows 0:64 = first half of head dim."""
            qf = rpool.tile([P, CH], BF, tag="qf")
            nc.vector.tensor_copy(out=qf, in_=ps)
            qs = rpool.tile([P, CH], BF, tag="qs")
            nc.sync.dma_start(out=qs[0:64, :], in_=qf[64:128, :])
            nc.sync.dma_start(out=qs[64:128, :], in_=qf[0:64, :])
            nc.vector.tensor_mul(out=qf, in0=qf, in1=cos_sb[:, c0:c1])
            nc.vector.tensor_mul(out=qs, in0=qs, in1=sin_sb[:, c0:c1])
            nc.vector.tensor_add(out=dst, in0=qf, in1=qs)

        for b in range(B):
            # ---------- phase P: projections + RoPE ----------
            qT = qpool.tile([P, HPC, S_], BF)
            kT = kpool.tile([P, S_], BF)
            v = vpool.tile([P, NKT, 132], BF)
            nc.vector.memset(v[:, :, 128:129], 1.0)
            for t in range(NCH):
                c0 = b * S_ + t * CH
                c1 = c0 + CH
                xt = xpool.tile([P, NKK, CH], BF)
                nc.sync.dma_start(out=xt, in_=xT_r[:, :, c0:c1])
                for g in range(HPC):
                    ps = psA.tile([P, CH], F32, tag="pj")
                    for kk in range(NKK):
                        nc.tensor.matmul(
                            ps,
                            lhsT=wq_sb[:, kk, g * HD:(g + 1) * HD],
                            rhs=xt[:, kk, :],
                            start=(kk == 0),
                            stop=(kk == NKK - 1),
                        )
                    rope(ps, qT[:, g, t * CH:t * CH + CH], c0, c1)
                ps = psA.tile([P, CH], F32, tag="pj")
                for kk in range(NKK):
                    nc.tensor.matmul(
                        ps, lhsT=wk_sb[:, kk, :], rhs=xt[:, kk, :],
                        start=(kk == 0), stop=(kk == NKK - 1),
                    )
                rope(ps, kT[:, t * CH:t * CH + CH], c0, c1)
                for ss in range(NVS):
                    pv = psV.tile([P, P], F32, tag="v")
                    for kk in range(NKK):
                        nc.tensor.matmul(
                            pv,
                            lhsT=xt[:, kk, ss * P:(ss + 1) * P],
                            rhs=wv_sb[:, kk, :],
                            start=(kk == 0), stop=(kk == NKK - 1),
                        )
                    nc.vector.tensor_copy(out=v[:, t * NVS + ss, 0:P], in_=pv)

            # ---------- phase A: attention ----------
            ctxT = ctpool.tile([P, HPC, S_], BF)
            for h in range(HPC):
                for qc in range(NQC):
                    nkt = (qc + 1) * NST
                    pcs = []
                    for _ in range(NST):
                        pc = psC.tile([P, 132], F32, tag="ctx")
                        pcs.append(pc)
                    for kt in range(nkt):
                        pss = psS.tile([P, QC], F32, tag="s")
                        nc.tensor.matmul(
                            pss,
                            lhsT=kT[:, kt * P:(kt + 1) * P],
                            rhs=qT[:, h, qc * QC:(qc + 1) * QC],
                            start=True, stop=True,
                        )
                        d = kt - qc * NST
                        if 0 <= d < NST:
                            nc.vector.tensor_add(
                                out=pss[:, d * P:(d + 1) * P],
                                in0=pss[:, d * P:(d + 1) * P],
                                in1=mneg_sb,
                            )
                        pb = ppool.tile([P, QC], BF)
                        nc.scalar.activation(out=pb, in_=pss, func=Exp, scale=SCALE)
                        for st in range(NST):
                            qsi = qc * NST + st
                            if kt <= qsi:
                                nc.tensor.matmul(
                                    pcs[st][:, 0:129],
                                    lhsT=pb[:, st * P:(st + 1) * P],
                                    rhs=v[:, kt, 0:129],
                                    start=(kt == 0), stop=(kt == qsi),
                                )
                    for st in range(NST):
                        qsi = qc * NST + st
                        rc = scpool.tile([P, 1], F32)
                        nc.vector.reciprocal(out=rc, in_=pcs[st][:, 128:129])
                        cn = cnpool.tile([P, P], BF)
                        nc.vector.tensor_scalar_mul(
                            out=cn, in0=pcs[st][:, 0:P], scalar1=rc
                        )
                        pt = psV.tile([P, P], BF, tag="v")
                        nc.tensor.transpose(pt, cn, iden_sb)
                        nc.scalar.copy(
                            out=ctxT[:, h, qsi * P:(qsi + 1) * P], in_=pt
                        )

            # ---------- phase W: output projection (partial) ----------
            for hc in range(HOC):
                wot = wopool.tile([P, HPC, OCW], BF)
                nc.sync.dma_start(out=wot, in_=wo_r[:, :, hc * OCW:(hc + 1) * OCW])
                for si in range(S_ // P):
                    po = psS.tile([P, OCW], F32, tag="s")
                    for ot in range(HPC):
                        nc.tensor.matmul(
                            po,
                            lhsT=ctxT[:, ot, si * P:(si + 1) * P],
                            rhs=wot[:, ot, :],
                            start=(ot == 0), stop=(ot == HPC - 1),
                        )
                    ob = opool.tile([P, OCW], BF)
                    nc.vector.tensor_copy(out=ob, in_=po)
                    nc.sync.dma_start(
                        out=out_h[b * S_ + si * P: b * S_ + (si + 1) * P,
                                  hc * OCW:(hc + 1) * OCW],
                        in_=ob,
                    )

    nc.compile()
    return nc


_CACHE = {}


def _get_graph():
    if "nc" not in _CACHE:
        _CACHE["nc"] = build_graph()
    return _CACHE["nc"]


def _host_prep(hidden_states, positions, Wq, Wk, Wv, Wo):
    """Transpose/cast/slice inputs per core. Returns list of 8 input dicts."""
    x2 = np.ascontiguousarray(hidden_states.reshape(T, HID).T).astype(BF16NP)

    pos = positions.astype(np.float32)                      # [B, S]
    half = HD // 2
    inv_freq = 1.0 / (ROPE_BASE ** (np.arange(half, dtype=np.float32) / half))
    ang = pos[:, :, None] * inv_freq[None, None, :]         # [B, S, 64]
    cos = np.cos(ang)
    sin = np.sin(ang)
    cosT = np.concatenate([cos[b].T for b in range(B)], axis=1)   # [64, T]
    sinT = np.concatenate([sin[b].T for b in range(B)], axis=1)
    cos2 = np.concatenate([cosT, cosT], axis=0).astype(BF16NP)    # [128, T]
    sin2 = np.concatenate([-sinT, sinT], axis=0).astype(BF16NP)

    r = np.arange(P)
    mneg = np.where(r[:, None] <= r[None, :], 0.0, -1e30).astype(np.float32)
    iden = np.eye(P, dtype=np.float32).astype(BF16NP)

    in_maps = []
    for c in range(NCORES):
        qs = slice(c * HPC * HD, (c + 1) * HPC * HD)
        ks = slice(c * HD, (c + 1) * HD)
        in_maps.append({
            "xT": x2,
            "wqT": np.ascontiguousarray(Wq[qs, :].T).astype(BF16NP),
            "wkT": np.ascontiguousarray(Wk[ks, :].T).astype(BF16NP),
            "wvT": np.ascontiguousarray(Wv[ks, :].T).astype(BF16NP),
            "woT": np.ascontiguousarray(Wo[:, qs].T).astype(BF16NP),
            "cos2": cos2,
            "sin2": sin2,
            "mneg": mneg,
            "iden": iden,
        })
    return in_maps


def kernel(hidden_states, positions, Wq, Wk, Wv, Wo):
    from concourse.bass_utils import run_bass_kernel_spmd

    trace = bool(os.environ.get("CLAUDE_KERNEL_TRACE"))
    if trace:
        _install_ntff_hook()

    nc = _get_graph()
    in_maps = _host_prep(
        np.asarray(hidden_states), np.asarray(positions),
        np.asarray(Wq), np.asarray(Wk), np.asarray(Wv), np.asarray(Wo),
    )
    res = run_bass_kernel_spmd(
        nc, in_maps, core_ids=list(range(NCORES)), trace=trace,
    )
    LAST["exec_time_ns"] = res.exec_time_ns
    LAST["profile_json"] = res.profile_json
    if res.instructions_and_trace is not None:
        LAST["trace_path"] = res.instructions_and_trace[1]

    acc = np.zeros((T, HID), np.float32)
    for c in range(NCORES):
        acc += res.results[c]["out"].astype(np.float32)
    return acc.reshape(B, S, HID)


# revision 5
# speedup vs baseline: 1.1096x; 1.1096x over previous
"""Trainium2 Bass kernel for GQA attention layer (RoPE + causal + GQA 32q/8kv).

Self-contained: hardcodes shapes from the problem spec.
  hidden_states [2, 2048, 4096] f32, positions [2, 2048] i32,
  Wq [4096, 4096], Wk [1024, 4096], Wv [1024, 4096], Wo [4096, 4096]  (all f32)
Sharding: tensor-parallel over heads across 8 cores. Core c gets kv head c and
q heads 4c..4c+3. Each core computes its partial Wo output; host sums partials.
"""

import math
import os
import sys
import types
import contextlib

import numpy as np
import ml_dtypes

BF16NP = ml_dtypes.bfloat16

# ---- problem constants (hardcoded per spec) ----
P = 128
B = 2
S = 2048            # tokens per batch
HID = 4096
NH, NKV, HD = 32, 8, 128
NCORES = 8
HPC = NH // NCORES  # q heads per core (4)
T = B * S
SCALE = 1.0 / math.sqrt(HD)
ROPE_BASE = 10000.0

LAST = {}           # exec_time_ns etc from the most recent run


def _install_ntff_hook():
    """Register the axon NTFF profiling hook (image's antenv lacks axon_hooks)."""
    if "antenv.axon_hooks" in sys.modules:
        return
    try:
        import antenv
        mod = types.ModuleType("antenv.axon_hooks")
        _box = [None]
        mod.set_axon_ntff_profile_hook = lambda h: _box.__setitem__(0, h)
        mod.get_axon_ntff_profile_hook = lambda: _box[0]
        sys.modules["antenv.axon_hooks"] = mod
        antenv.axon_hooks = mod
        from trn_agent_boot.trn_boot import _ntff_profile_via_ctypes
        mod.set_axon_ntff_profile_hook(
            _ntff_profile_via_ctypes("/opt/axon/libaxon_pjrt.so")
        )
    except Exception:
        pass


def build_graph(S_=S, HID_=HID, CH=512, QC=512):
    """Build the per-core graph (identical on all 8 cores; SPMD via inputs).

    S_: tokens per batch, HID_: hidden size, CH: projection token chunk,
    QC: attention query chunk.
    """
    import concourse.bacc as bacc
    import concourse.mybir as mybir
    import concourse.tile as tile
    from contextlib import ExitStack

    BF = mybir.dt.bfloat16
    F32 = mybir.dt.float32
    Exp = mybir.ActivationFunctionType.Exp

    NKK = HID_ // P          # contraction tiles over hidden
    NCH = S_ // CH           # proj chunks per batch
    NQC = S_ // QC           # attention q chunks per batch
    NST = QC // P            # q subtiles per chunk
    NKT = S_ // P            # k tiles per batch
    NVS = CH // P            # v row-subtiles per proj chunk
    HOC = max(1, HID_ // 512)  # output column chunks
    OCW = min(512, HID_)       # output chunk width

    nc = bacc.Bacc(None)
    xT_h = nc.declare_dram_parameter("xT", [HID_, B * S_], BF, isOutput=False)
    wq_h = nc.declare_dram_parameter("wqT", [HID_, HPC * HD], BF, isOutput=False)
    wk_h = nc.declare_dram_parameter("wkT", [HID_, HD], BF, isOutput=False)
    wv_h = nc.declare_dram_parameter("wvT", [HID_, HD], BF, isOutput=False)
    wo_h = nc.declare_dram_parameter("woT", [HPC * HD, HID_], BF, isOutput=False)
    cos_h = nc.declare_dram_parameter("cos2", [P, B * S_], BF, isOutput=False)
    sin_h = nc.declare_dram_parameter("sin2", [P, B * S_], BF, isOutput=False)
    mneg_h = nc.declare_dram_parameter("mneg", [P, P], F32, isOutput=False)
    iden_h = nc.declare_dram_parameter("iden", [P, P], BF, isOutput=False)
    out_h = nc.declare_dram_parameter("out", [B * S_, HID_], BF, isOutput=True)

    xT_r = xT_h[:, :].rearrange("(ko ki) s -> ki ko s", ki=P)
    wq_r = wq_h[:, :].rearrange("(ko ki) d -> ki ko d", ki=P)
    wk_r = wk_h[:, :].rearrange("(ko ki) d -> ki ko d", ki=P)
    wv_r = wv_h[:, :].rearrange("(ko ki) d -> ki ko d", ki=P)
    wo_r = wo_h[:, :].rearrange("(oo oi) h -> oi oo h", oi=P)

    with tile.TileContext(nc) as tc, ExitStack() as ctx:
        wpool = ctx.enter_context(tc.tile_pool(name="wpool", bufs=1))
        qpool = ctx.enter_context(tc.tile_pool(name="qpool", bufs=2))
        kpool = ctx.enter_context(tc.tile_pool(name="kpool", bufs=2))
        vpool = ctx.enter_context(tc.tile_pool(name="vpool", bufs=2))
        ctpool = ctx.enter_context(tc.tile_pool(name="ctpool", bufs=2))
        xpool = ctx.enter_context(tc.tile_pool(name="xpool", bufs=3))
        cspool = ctx.enter_context(tc.tile_pool(name="cspool", bufs=2))
        vtpool = ctx.enter_context(tc.tile_pool(name="vtpool", bufs=2))
        wopool = ctx.enter_context(tc.tile_pool(name="wopool", bufs=2))
        ppool = ctx.enter_context(tc.tile_pool(name="ppool", bufs=3))
        rpool = ctx.enter_context(tc.tile_pool(name="rpool", bufs=3))
        cnpool = ctx.enter_context(tc.tile_pool(name="cnpool", bufs=3))
        opool = ctx.enter_context(tc.tile_pool(name="opool", bufs=3))
        scpool = ctx.enter_context(tc.tile_pool(name="scpool", bufs=4))

        psX = ctx.enter_context(tc.tile_pool(name="psX", bufs=6, space="PSUM"))
        psS = ctx.enter_context(tc.tile_pool(name="psS", bufs=2, space="PSUM"))

        # --- persistent weights / tables ---
        wq_sb = wpool.tile([P, NKK, HPC * HD], BF)
        nc.sync.dma_start(out=wq_sb, in_=wq_r)
        wk_sb = wpool.tile([P, NKK, HD], BF)
        nc.sync.dma_start(out=wk_sb, in_=wk_r)
        wv_sb = wpool.tile([P, NKK, HD], BF)
        nc.sync.dma_start(out=wv_sb, in_=wv_r)
        mneg_sb = wpool.tile([P, P], F32)
        nc.sync.dma_start(out=mneg_sb, in_=mneg_h[:, :])
        iden_sb = wpool.tile([P, P], BF)
        nc.sync.dma_start(out=iden_sb, in_=iden_h[:, :])

        def rope(ps, dst, cs, sn):
            """Neox RoPE on [128 d, n] tile: rows 0:64 = first half of head dim."""
            qf = rpool.tile([P, CH], BF, tag="qf")
            nc.vector.tensor_copy(out=qf, in_=ps)
            qs = rpool.tile([P, CH], BF, tag="qs")
            nc.sync.dma_start(out=qs[0:64, :], in_=qf[64:128, :])
            nc.sync.dma_start(out=qs[64:128, :], in_=qf[0:64, :])
            nc.vector.tensor_mul(out=qf, in0=qf, in1=cs)
            nc.vector.tensor_mul(out=qs, in0=qs, in1=sn)
            nc.vector.tensor_add(out=dst, in0=qf, in1=qs)

        for b in range(B):
            # ---------- phase P: projections + RoPE ----------
            qT = qpool.tile([P, HPC, S_], BF)
            kT = kpool.tile([P, S_], BF)
            v = vpool.tile([P, NKT, 132], BF)
            nc.vector.memset(v[:, :, 128:129], 1.0)
            NKH = NKK // 2
            for t in range(NCH):
                c0 = b * S_ + t * CH
                c1 = c0 + CH
                xta = xpool.tile([P, NKH, CH], BF, tag="x")
                nc.sync.dma_start(out=xta, in_=xT_r[:, 0:NKH, c0:c1])
                xtb = xpool.tile([P, NKH, CH], BF, tag="x")
                nc.sync.dma_start(out=xtb, in_=xT_r[:, NKH:NKK, c0:c1])
                cs = cspool.tile([P, CH], BF, tag="cos")
                nc.sync.dma_start(out=cs, in_=cos_h[:, c0:c1])
                sn = cspool.tile([P, CH], BF, tag="sin")
                nc.sync.dma_start(out=sn, in_=sin_h[:, c0:c1])

                def xt(kk):
                    return xta[:, kk, :] if kk < NKH else xtb[:, kk - NKH, :]

                for g in range(HPC):
                    ps = psX.tile([P, CH], F32, tag="px")
                    for kk in range(NKK):
                        nc.tensor.matmul(
                            ps,
                            lhsT=wq_sb[:, kk, g * HD:(g + 1) * HD],
                            rhs=xt(kk),
                            start=(kk == 0),
                            stop=(kk == NKK - 1),
                        )
                    rope(ps, qT[:, g, t * CH:t * CH + CH], cs, sn)
                ps = psX.tile([P, CH], F32, tag="px")
                for kk in range(NKK):
                    nc.tensor.matmul(
                        ps, lhsT=wk_sb[:, kk, :], rhs=xt(kk),
                        start=(kk == 0), stop=(kk == NKK - 1),
                    )
                rope(ps, kT[:, t * CH:t * CH + CH], cs, sn)
                # V in vT orientation (N=CH matmuls), then PE-transpose to [s, d]
                pv = psX.tile([P, CH], F32, tag="px")
                for kk in range(NKK):
                    nc.tensor.matmul(
                        pv, lhsT=wv_sb[:, kk, :], rhs=xt(kk),
                        start=(kk == 0), stop=(kk == NKK - 1),
                    )
                vt = vtpool.tile([P, CH], BF, tag="vt")
                nc.vector.tensor_copy(out=vt, in_=pv)
                for ss in range(NVS):
                    pq = psX.tile([P, P], BF, tag="px")
                    nc.tensor.transpose(pq, vt[:, ss * P:(ss + 1) * P], iden_sb)
                    nc.scalar.copy(out=v[:, t * NVS + ss, 0:P], in_=pq)

            # ---------- phase A: attention ----------
            ctxT = ctpool.tile([P, HPC, S_], BF)
            for h in range(HPC):
                for qc in range(NQC):
                    nkt = (qc + 1) * NST
                    pcs = []
                    for _ in range(NST):
                        pc = psX.tile([P, 132], F32, tag="px")
                        pcs.append(pc)
                    for kt in range(nkt):
                        pss = psS.tile([P, QC], F32, tag="s")
                        nc.tensor.matmul(
                            pss,
                            lhsT=kT[:, kt * P:(kt + 1) * P],
                            rhs=qT[:, h, qc * QC:(qc + 1) * QC],
                            start=True, stop=True,
                        )
                        d = kt - qc * NST
                        if 0 <= d < NST:
                            nc.vector.tensor_add(
                                out=pss[:, d * P:(d + 1) * P],
                                in0=pss[:, d * P:(d + 1) * P],
                                in1=mneg_sb,
                            )
                        pb = ppool.tile([P, QC], BF)
                        nc.scalar.activation(out=pb, in_=pss, func=Exp, scale=SCALE)
                        for st in range(NST):
                            qsi = qc * NST + st
                            if kt <= qsi:
                                nc.tensor.matmul(
                                    pcs[st][:, 0:129],
                                    lhsT=pb[:, st * P:(st + 1) * P],
                                    rhs=v[:, kt, 0:129],
                                    start=(kt == 0), stop=(kt == qsi),
                                )
                    for st in range(NST):
                        qsi = qc * NST + st
                        rc = scpool.tile([P, 1], F32)
                        nc.vector.reciprocal(out=rc, in_=pcs[st][:, 128:129])
                        cn = cnpool.tile([P, P], BF)
                        nc.vector.tensor_scalar_mul(
                            out=cn, in0=pcs[st][:, 0:P], scalar1=rc
                        )
                        pt = psX.tile([P, P], BF, tag="px")
                        nc.tensor.transpose(pt, cn, iden_sb)
                        nc.scalar.copy(
                            out=ctxT[:, h, qsi * P:(qsi + 1) * P], in_=pt
                        )

            # ---------- phase W: output projection (partial) ----------
            for hc in range(HOC):
                wot = wopool.tile([P, HPC, OCW], BF)
                nc.sync.dma_start(out=wot, in_=wo_r[:, :, hc * OCW:(hc + 1) * OCW])
                for si in range(S_ // P):
                    po = psS.tile([P, OCW], F32, tag="s")
                    for ot in range(HPC):
                        nc.tensor.matmul(
                            po,
                            lhsT=ctxT[:, ot, si * P:(si + 1) * P],
                            rhs=wot[:, ot, :],
                            start=(ot == 0), stop=(ot == HPC - 1),
                        )
                    ob = opool.tile([P, OCW], BF)
                    if (si + hc) % 2 == 0:
                        nc.vector.tensor_copy(out=ob, in_=po)
                    else:
                        nc.scalar.copy(out=ob, in_=po)
                    nc.sync.dma_start(
                        out=out_h[b * S_ + si * P: b * S_ + (si + 1) * P,
                                  hc * OCW:(hc + 1) * OCW],
                        in_=ob,
                    )

    nc.compile()
    return nc


_CACHE = {}


def _get_graph():
    if "nc" not in _CACHE:
        _CACHE["nc"] = build_graph()
    return _CACHE["nc"]


def _host_prep(hidden_states, positions, Wq, Wk, Wv, Wo):
    """Transpose/cast/slice inputs per core. Returns list of 8 input dicts."""
    x2 = np.ascontiguousarray(hidden_states.reshape(T, HID).T).astype(BF16NP)

    pos = positions.astype(np.float32)                      # [B, S]
    half = HD // 2
    inv_freq = 1.0 / (ROPE_BASE ** (np.arange(half, dtype=np.float32) / half))
    ang = pos[:, :, None] * inv_freq[None, None, :]         # [B, S, 64]
    cos = np.cos(ang)
    sin = np.sin(ang)
    cosT = np.concatenate([cos[b].T for b in range(B)], axis=1)   # [64, T]
    sinT = np.concatenate([sin[b].T for b in range(B)], axis=1)
    cos2 = np.concatenate([cosT, cosT], axis=0).astype(BF16NP)    # [128, T]
    sin2 = np.concatenate([-sinT, sinT], axis=0).astype(BF16NP)

    r = np.arange(P)
    mneg = np.where(r[:, None] <= r[None, :], 0.0, -1e30).astype(np.float32)
    iden = np.eye(P, dtype=np.float32).astype(BF16NP)

    in_maps = []
    for c in range(NCORES):
        qs = slice(c * HPC * HD, (c + 1) * HPC * HD)
        ks = slice(c * HD, (c + 1) * HD)
        in_maps.append({
            "xT": x2,
            "wqT": np.ascontiguousarray(Wq[qs, :].T).astype(BF16NP),
            "wkT": np.ascontiguousarray(Wk[ks, :].T).astype(BF16NP),
            "wvT": np.ascontiguousarray(Wv[ks, :].T).astype(BF16NP),
            "woT": np.ascontiguousarray(Wo[:, qs].T).astype(BF16NP),
            "cos2": cos2,
            "sin2": sin2,
            "mneg": mneg,
            "iden": iden,
        })
    return in_maps


def kernel(hidden_states, positions, Wq, Wk, Wv, Wo):
    from concourse.bass_utils import run_bass_kernel_spmd

    trace = bool(os.environ.get("CLAUDE_KERNEL_TRACE"))
    if trace:
        _install_ntff_hook()

    nc = _get_graph()
    in_maps = _host_prep(
        np.asarray(hidden_states), np.asarray(positions),
        np.asarray(Wq), np.asarray(Wk), np.asarray(Wv), np.asarray(Wo),
    )
    res = run_bass_kernel_spmd(
        nc, in_maps, core_ids=list(range(NCORES)), trace=trace,
    )
    LAST["exec_time_ns"] = res.exec_time_ns
    LAST["profile_json"] = res.profile_json
    if res.instructions_and_trace is not None:
        LAST["trace_path"] = res.instructions_and_trace[1]

    acc = np.zeros((T, HID), np.float32)
    for c in range(NCORES):
        acc += res.results[c]["out"].astype(np.float32)
    return acc.reshape(B, S, HID)
